# revision 1
# baseline (speedup 1.0000x reference)
"""Trainium2 Bass kernel for LogWeightedDICELossMultiClass3D.

Input: output (4,3,64,192,192) f32, masks (same), loss_threshold scalar.
Strategy: shard H=192 into 8 slabs of 24 rows (one per NeuronCore, with a
1-row halo clamped on host). Each core reduces its slab of all 12 (b,c)
volumes to 5 per-(volume,z) partial sums:
  sum(m), sum(o), sum(o*m), sum((o>thr)==m), sum(sobel_edge)
The host combines the tiny partials (12 values per quantity) into the loss.

Device layout per core: 6 supertiles of 128 partitions = 2 volumes x 64 z.
Free dim = 26 H-rows (24 + 2 halo) x 192 W, flat (4992 f32).
The 3D sobel is computed as W-derivative (DVE, bf16, exact small ints)
followed by Z-smoothing on the PE (block-diag banded matmul) with the
H-smoothing folded in as 3 H-shifted accumulating matmuls (weights B,B,2B).
grad is integer-valued, so edge=(grad>0) is computed exactly by a saturated
Sigmoid(100*grad-50) on the ACT engine with a fused per-partition reduction.
"""

import numpy as np
import ml_dtypes

import concourse.bacc as bacc
import concourse.bass as bass
import concourse.tile as tile
from concourse import mybir
from concourse.bass_utils import run_bass_kernel_spmd

F32 = mybir.dt.float32
BF16 = mybir.dt.bfloat16
ALU = mybir.AluOpType
ACTF = mybir.ActivationFunctionType

B, C, Z, H, W = 4, 3, 64, 192, 192
NV = B * C            # 12 volumes
NCORES = 8
HC = H // NCORES      # 24 H-rows per core
NS = NV // 2          # 6 supertiles (2 volumes each)
FH = HC + 2           # 26 rows incl halo
FW = FH * W           # 4992 free elements per partition (o / ts / d)
CW = HC * W           # 4608 center free elements (m / eq / om)
C0 = W                # flat offset of center region (row 1)
SPAN = 1536           # grad span (3 per supertile), 3 psum banks
VOX = Z * H * W

_CACHE = {}


def _band64():
    """[1,2,1] smoothing matrix with scipy 'reflect' (np symmetric) ends."""
    M = np.zeros((Z, Z), dtype=np.float64)
    for i in range(Z):
        M[i, i] = 2.0
        if i > 0:
            M[i, i - 1] += 1.0
        else:
            M[i, i] += 1.0
        if i < Z - 1:
            M[i, i + 1] += 1.0
        else:
            M[i, i] += 1.0
    return M


def _consts():
    Bz = _band64()
    blk = np.zeros((128, 128), dtype=np.float64)
    blk[:64, :64] = Bz
    blk[64:, 64:] = Bz
    bz1 = blk.astype(ml_dtypes.bfloat16)          # weights 1,2,3 - exact
    bz2 = (2.0 * blk).astype(ml_dtypes.bfloat16)  # weights 2,4,6 - exact
    ones2 = np.zeros((128, 2), dtype=np.float32)
    ones2[:64, 0] = 1.0
    ones2[64:, 1] = 1.0
    return bz1, bz2, ones2


def _build_program():
    import os
    stage = int(os.environ.get("KERNEL_STAGE", "99"))
    nc = bacc.Bacc("TRN2", target_bir_lowering=False, debug=False,
                   num_devices=NCORES)
    o_d = nc.dram_tensor("o", [NV * Z, FW], F32, kind="ExternalInput").ap()
    m_d = nc.dram_tensor("m", [NV * Z, CW], F32, kind="ExternalInput").ap()
    thr_d = nc.dram_tensor("thr", [1, 1], F32, kind="ExternalInput").ap()
    bz1_d = nc.dram_tensor("bz1", [128, 128], BF16, kind="ExternalInput").ap()
    bz2_d = nc.dram_tensor("bz2", [128, 128], BF16, kind="ExternalInput").ap()
    ones_d = nc.dram_tensor("ones2", [128, 2], F32, kind="ExternalInput").ap()
    part_d = nc.dram_tensor("partials", [128, 36], F32, kind="ExternalOutput").ap()
    osum_d = nc.dram_tensor("osum", [2, 512 * NS], F32, kind="ExternalOutput").ap()

    from contextlib import ExitStack
    with tile.TileContext(nc) as tc, ExitStack() as ctx:
        consts = ctx.enter_context(tc.tile_pool(name="consts", bufs=1))
        io = ctx.enter_context(tc.tile_pool(name="io", bufs=2))
        mid = ctx.enter_context(tc.tile_pool(name="mid", bufs=2))
        slots = ctx.enter_context(tc.tile_pool(name="slots", bufs=1))
        if stage >= 1:
            scr = ctx.enter_context(tc.tile_pool(name="scr", bufs=2))
        if stage >= 4:
            ops = ctx.enter_context(tc.tile_pool(name="ops", bufs=2, space="PSUM"))
        if stage >= 5:
            gps = ctx.enter_context(tc.tile_pool(name="gps", bufs=2, space="PSUM"))

        thr_t = consts.tile([128, 1], F32)
        nc.gpsimd.dma_start(out=thr_t, in_=thr_d.to_broadcast([128, 1]))
        bz1_t = consts.tile([128, 128], BF16)
        nc.default_dma_engine.dma_start(out=bz1_t, in_=bz1_d)
        bz2_t = consts.tile([128, 128], BF16)
        nc.default_dma_engine.dma_start(out=bz2_t, in_=bz2_d)
        ones_t = consts.tile([128, 2], F32)
        nc.default_dma_engine.dma_start(out=ones_t, in_=ones_d)
        nbias_t = consts.tile([128, 1], F32)
        nc.vector.memset(nbias_t, -50.0)
        if stage < 5:
            # keep bz/ones/nbias tiles "used" in reduced builds
            nc.vector.tensor_copy(bz2_t, bz1_t)
            nc.vector.tensor_copy(ones_t[:, 0:1], nbias_t)

        msum = slots.tile([128, NS], F32)
        eqsum = slots.tile([128, NS], F32)
        omsum = slots.tile([128, NS], F32)
        edgesum = slots.tile([128, 3 * NS], F32)
        osb = slots.tile([2, 512 * NS], F32)
        if stage < 99:
            nc.vector.memset(eqsum, 0.0)
            nc.vector.memset(omsum, 0.0)
            nc.vector.memset(edgesum, 0.0)
            nc.vector.memset(osb, 0.0)

        for s in range(NS):
            o_t = io.tile([128, FW], F32, tag="o", name=f"o{s}")
            nc.default_dma_engine.dma_start(
                out=o_t, in_=o_d[128 * s:128 * (s + 1), :])
            m_t = io.tile([128, CW], F32, tag="m", name=f"m{s}")
            nc.default_dma_engine.dma_start(
                out=m_t, in_=m_d[128 * s:128 * (s + 1), :])

            # ts = (o > thr), bf16 0/1 (exact)
            ts_t = mid.tile([128, FW], BF16, tag="ts", name=f"ts{s}")
            nc.vector.tensor_scalar(
                out=ts_t, in0=o_t, scalar1=thr_t, scalar2=None,
                op0=ALU.is_gt)

            # m cast to bf16 on ACT + fused per-partition sum(m)
            m_b = mid.tile([128, CW], BF16, tag="mb", name=f"mb{s}")
            nc.scalar.activation(
                out=m_b, in_=m_t, func=ACTF.Copy,
                accum_out=msum[:, s:s + 1])

            if stage >= 1:
                # eq = (ts == m) with fused sum
                scr_t = scr.tile([128, CW], BF16, tag="scr", name=f"scr{s}")
                nc.vector.scalar_tensor_tensor(
                    out=scr_t, in0=ts_t[:, C0:C0 + CW], scalar=0.0, in1=m_b,
                    op0=ALU.add, op1=ALU.is_equal,
                    accum_out=eqsum[:, s:s + 1])
            else:
                nc.vector.tensor_copy(m_b[:, 0:1], ts_t[:, 0:1])

            if stage >= 2:
                # om = o * m with fused sum
                om_t = scr.tile([128, CW], BF16, tag="scr2", name=f"om{s}")
                nc.vector.scalar_tensor_tensor(
                    out=om_t, in0=o_t[:, C0:C0 + CW], scalar=0.0, in1=m_t,
                    op0=ALU.add, op1=ALU.mult,
                    accum_out=omsum[:, s:s + 1])

            if stage >= 3:
                # d = W-derivative of ts (symmetric boundary), cols permuted:
                # cols 0..189 = d[w=1..190], col 190 = d[w=0], col 191 = d[w=191]
                d_t = mid.tile([128, FW], BF16, tag="d", name=f"d{s}")
                ts3 = ts_t.rearrange("p (a b) -> p a b", b=W)
                d3 = d_t.rearrange("p (a b) -> p a b", b=W)
                nc.vector.tensor_tensor(
                    out=d3[:, :, 0:190], in0=ts3[:, :, 2:192],
                    in1=ts3[:, :, 0:190], op=ALU.subtract)
                nc.vector.tensor_tensor(
                    out=d3[:, :, 190:191], in0=ts3[:, :, 1:2],
                    in1=ts3[:, :, 0:1], op=ALU.subtract)
                nc.vector.tensor_tensor(
                    out=d3[:, :, 191:192], in0=ts3[:, :, 191:192],
                    in1=ts3[:, :, 190:191], op=ALU.subtract)

            if stage >= 4:
                # sum(o) over center via ones-stationary matmul (PE)
                op_t = ops.tile([2, 512], F32, tag="op", name=f"op{s}")
                for k in range(9):
                    nc.tensor.matmul(
                        out=op_t,
                        lhsT=ones_t,
                        rhs=o_t[:, C0 + 512 * k:C0 + 512 * (k + 1)],
                        start=(k == 0), stop=(k == 8))
                nc.scalar.copy(out=osb[:, 512 * s:512 * (s + 1)], in_=op_t)

            if stage >= 5:
                # grad = S_Z(S_H(d)) via 3 H-shifted banded matmuls into PSUM
                for j in range(3):
                    g_t = gps.tile([128, SPAN], F32, tag="g", name=f"g{s}_{j}")
                    base = C0 + SPAN * j
                    for di, (lhs, doff) in enumerate(
                            [(bz1_t, -W), (bz1_t, W), (bz2_t, 0)]):
                        for k in range(3):
                            off = base + 512 * k + doff
                            nc.tensor.matmul(
                                out=g_t[:, 512 * k:512 * (k + 1)],
                                lhsT=lhs, rhs=d_t[:, off:off + 512],
                                start=(di == 0), stop=(di == 2))
                    # edge = (grad > 0): integer grad, sigmoid saturates
                    e_t = scr.tile([128, SPAN], BF16, tag="edge",
                                   name=f"e{s}_{j}")
                    nc.scalar.activation(
                        out=e_t, in_=g_t, func=ACTF.Sigmoid,
                        scale=100.0, bias=nbias_t,
                        accum_out=edgesum[:, 3 * s + j:3 * s + j + 1])

        nc.default_dma_engine.dma_start(out=part_d[:, 0:6], in_=msum)
        nc.default_dma_engine.dma_start(out=part_d[:, 6:12], in_=eqsum)
        nc.default_dma_engine.dma_start(out=part_d[:, 12:18], in_=omsum)
        nc.default_dma_engine.dma_start(out=part_d[:, 18:36], in_=edgesum)
        nc.default_dma_engine.dma_start(out=osum_d, in_=osb)

    nc.compile()
    return nc


def _get_program():
    if "nc" not in _CACHE:
        _CACHE["nc"] = _build_program()
    return _CACHE["nc"]


def _make_in_maps(output, masks, loss_threshold):
    o5 = np.ascontiguousarray(np.asarray(output, dtype=np.float32)).reshape(
        NV, Z, H, W)
    m5 = np.ascontiguousarray(np.asarray(masks, dtype=np.float32)).reshape(
        NV, Z, H, W)
    thr = np.full((1, 1), np.float32(np.asarray(loss_threshold)), np.float32)
    bz1, bz2, ones2 = _consts()
    in_maps = []
    for c in range(NCORES):
        h0 = HC * c
        idx = np.clip(np.arange(h0 - 1, h0 + HC + 1), 0, H - 1)
        o_sh = np.ascontiguousarray(o5[:, :, idx, :]).reshape(NV * Z, FW)
        m_sh = np.ascontiguousarray(m5[:, :, h0:h0 + HC, :]).reshape(NV * Z, CW)
        in_maps.append({
            "o": o_sh, "m": m_sh, "thr": thr,
            "bz1": bz1, "bz2": bz2, "ones2": ones2,
        })
    return in_maps


def _combine(results):
    """Host-side tiny reduction: per-core partials -> loss scalar."""
    sum_m = np.zeros(NV)
    sum_eq = np.zeros(NV)
    sum_om = np.zeros(NV)
    sum_edge = np.zeros(NV)
    sum_o = np.zeros(NV)
    for r in results:
        p = np.asarray(r["partials"], dtype=np.float64)
        osum = np.asarray(r["osum"], dtype=np.float64)
        # [p, s]: volume = 2s + p//64, z = p%64
        sum_m += p[:, 0:6].reshape(2, 64, NS).sum(1).T.reshape(-1)
        sum_eq += p[:, 6:12].reshape(2, 64, NS).sum(1).T.reshape(-1)
        sum_om += p[:, 12:18].reshape(2, 64, NS).sum(1).T.reshape(-1)
        sum_edge += (p[:, 18:36].reshape(2, 64, NS, 3).sum(axis=(1, 3))
                     .T.reshape(-1))
        sum_o += osum.reshape(2, NS, 512).sum(-1).T.reshape(-1)

    freq = (sum_m / VOX).reshape(B, C)
    med = np.median(freq, axis=1, keepdims=True)
    w0 = 2.0 * med / (freq.min(axis=1, keepdims=True) + 1e-5)
    cw = (med / (freq + 1e-5)) * sum_eq.reshape(B, C) \
        + w0 * sum_edge.reshape(B, C)
    ps1 = sum_om.reshape(B, C)
    ps2 = (sum_o + sum_m).reshape(B, C)
    nom = (cw * ps1).sum(1)
    denom = (cw * ps2 + 1e-7).sum(1)
    loss = (1.0 - 2.0 * nom / denom).sum() / B
    return np.array([loss], dtype=np.float32)


def run(output, masks, loss_threshold, trace=False, **trace_kwargs):
    nc = _get_program()
    in_maps = _make_in_maps(output, masks, loss_threshold)
    res = run_bass_kernel_spmd(nc, in_maps, list(range(NCORES)),
                               trace=trace, **trace_kwargs)
    return _combine(res.results), res


def kernel(output, masks, loss_threshold):
    loss, _ = run(output, masks, loss_threshold)
    return loss

